# revision 1
# baseline (speedup 1.0000x reference)
"""Trainium2 Bass kernel for CausalSelfAttention (GQA + RMSNorm + partial RoPE).

Sharding: 8 cores = (batch b in 0..3) x (kv-head group g in 0..1).
Each core computes the full attention for its (b, g) slice and the partial
output projection over its head columns; the host sums the two partials per
batch and transposes back ([o, s] -> [s, o]).

Device layout strategy (per core, S=2048, D=1024, HD=128, 4 q heads, 1 kv):
  - QKV projections from xT [d, s] tiles; q produced in [s, o] layout for
    RMS-norm/RoPE (free-dim reductions), k/v produced in [hd, s] layout.
  - q is scaled by rstd_q * gain * HD^-0.5 and PE-transposed to qT [hd, s].
  - k's rstd is folded into the exp() per-partition scale during attention.
  - scoresT [sk, sq] = kT_slice.T @ qT; exp on ACT; causal handled by
    restricting diagonal matmuls to the valid sq range + one triangular
    multiplicative mask on the 128-wide diagonal block.
  - attn@v accumulates yT [hd, sq] in PSUM (v natural [sk, hd] stationary);
    softmax denominator accumulated on DVE/GPSIMD, reduced over partitions
    with a ones-matmul, reciprocal, broadcast back via a K=1 matmul, and
    multiplied into yT.
  - Output projection with w_o slices stationary -> outT [o, s] in DRAM.
"""

import sys

for _p in ("/opt/trn_rl_repo",):
    if _p not in sys.path:
        sys.path.insert(0, _p)

import numpy as np

import concourse.bass as bass
import concourse.bacc as bacc
import concourse.mybir as mybir
import concourse.tile as tile
from concourse import bass_utils
from concourse.masks import make_identity

F32 = mybir.dt.float32
F32R = mybir.dt.float32r
AFT = mybir.ActivationFunctionType

B, S, D = 4, 2048, 1024
H, KVH, HD = 8, 2, 128
NH = H // KVH          # q heads per core = 4
RD, RH = 64, 32        # rope dims / half
NB, BLK = 4, 512       # s blocks
NT, TS = 16, 128       # s tiles
NDC = D // 128         # 8 d-chunks
EPS = float(np.finfo(np.float32).eps)


# Heavy matmuls run in float32r (~12-bit mantissa, 1 cyc/row vs 4 for fp32).
# Operands must be produced by "rounding" instructions (ACT/DVE copies), so
# DMA-fed tensors get an on-chip rounding copy and compute-produced tensors
# are written with float32r output dtype directly.


def _build_nc(reps=1):
    nc = bacc.Bacc("TRN2", target_bir_lowering=False, debug=False,
                   enable_asserts=False)

    xT = nc.dram_tensor("xT", (D, S), F32, kind="ExternalInput").ap()
    wq = nc.dram_tensor("wq", (128, NDC, NH * HD), F32, kind="ExternalInput").ap()
    wkv = nc.dram_tensor("wkv", (128, NDC, 2 * HD), F32,
                         kind="ExternalInput").ap()
    wo = nc.dram_tensor("wo", (128, NH, D), F32, kind="ExternalInput").ap()
    cosq = nc.dram_tensor("cosq", (128, NT, RH), F32, kind="ExternalInput").ap()
    sinq = nc.dram_tensor("sinq", (128, NT, RH), F32, kind="ExternalInput").ap()
    nsinq = nc.dram_tensor("nsinq", (128, NT, RH), F32, kind="ExternalInput").ap()
    cosk2 = nc.dram_tensor("cosk2", (RD, S), F32, kind="ExternalInput").ap()
    sk2 = nc.dram_tensor("sk2", (RD, S), F32, kind="ExternalInput").ap()
    qsc = nc.dram_tensor("qsc", (1, NH), F32, kind="ExternalInput").ap()
    outT = nc.dram_tensor("outT", (D, S), F32, kind="ExternalOutput").ap()

    with tile.TileContext(nc) as tc, \
         nc.allow_low_precision(reason="float32r matmul operands"):
        for _rep in range(reps):
            _kern(nc, tc, xT, wq, wkv, wo, cosq, sinq, nsinq, cosk2, sk2,
                  qsc, outT)
    nc.compile()
    return nc


def _kern(nc, tc, xT, wq, wkv, wo, cosq, sinq, nsinq, cosk2, sk2, qsc,
          outT):
    mm = nc.tensor.matmul

    persist_cm = tc.tile_pool(name="persist", bufs=1)
    persist = persist_cm.__enter__()
    # ---- persistent tiles -------------------------------------------------
    wq_sb = persist.tile([128, NDC, NH * HD], F32R, tag="wq_sb", name="wq_sb")
    wkv_sb = persist.tile([128, NDC, 2 * HD], F32R, tag="wkv_sb",
                          name="wkv_sb")
    wo_sb = persist.tile([128, NH, D], F32R, tag="wo_sb", name="wo_sb")
    cosq_sb = persist.tile([128, NT, RH], F32, tag="cosq_sb", name="cosq_sb")
    nc.sync.dma_start(out=cosq_sb, in_=cosq)
    sinq_sb = persist.tile([128, NT, RH], F32, tag="sinq_sb", name="sinq_sb")
    nc.sync.dma_start(out=sinq_sb, in_=sinq)
    nsinq_sb = persist.tile([128, NT, RH], F32, tag="nsinq_sb", name="nsinq_sb")
    nc.sync.dma_start(out=nsinq_sb, in_=nsinq)
    qsc_sb = persist.tile([128, NH], F32, tag="qsc_sb", name="qsc_sb")
    nc.sync.dma_start(out=qsc_sb, in_=qsc.to_broadcast((128, NH)))

    ones_st = persist.tile([128, 128], F32, tag="ones_st", name="ones_st")
    nc.vector.memset(ones_st, 1.0)
    ones_all = persist.tile([128, 128], F32R, tag="ones_all", name="ones_all")
    nc.vector.tensor_copy(out=ones_all, in_=ones_st)
    ones_col = ones_all[:, 0:1]
    ones_row = ones_all[0:1, :]
    eps_col = persist.tile([128, 1], F32, tag="eps_col", name="eps_col")
    nc.vector.memset(eps_col, EPS)
    ident_st = persist.tile([128, 128], F32, tag="ident_st", name="ident_st")
    make_identity(nc, ident_st)
    ident = persist.tile([128, 128], F32R, tag="ident", name="ident")
    nc.vector.tensor_copy(out=ident, in_=ident_st)
    # tri[r, c] = 1.0 if r <= c else 0.0  (causal keep-mask on the diagonal)
    tri = persist.tile([128, 128], F32, tag="tri", name="tri")
    nc.gpsimd.memset(tri, 1.0)
    nc.gpsimd.affine_select(
        out=tri, in_=tri, compare_op=mybir.AluOpType.is_ge, fill=0.0,
        base=0, pattern=[[1, 128]], channel_multiplier=-1)

    qT_sb = persist.tile([128, NH, S], F32R, tag="qT_sb", name="qT_sb")      # [hd, h, s]
    kT_sb = persist.tile([128, S], F32R, tag="kT_sb", name="kT_sb")          # [hd, s]
    v_sb = persist.tile([128, NT, HD], F32R, tag="v_sb", name="v_sb")       # [sk, t, hd]
    rstdk_sb = persist.tile([128, NT], F32, tag="rstdk_sb", name="rstdk_sb")   # [sk, t]
    yT_sb = persist.tile([128, NH, S], F32R, tag="yT_sb", name="yT_sb")      # [hd, h, s]

    # ---- phase 1: projections + norm + rope + transposes ------------------
    # PSUM budget (8 banks): q_ps 4 + k_ps 2 + v_ps 1 + misc_ps 1.
    with tc.tile_pool(name="p1_psum", bufs=1, space="PSUM") as p1ps, \
         tc.tile_pool(name="p1_sbuf", bufs=1) as p1sb:
        for wdst, wsrc, wname in ((wq_sb, wq, "wq"), (wkv_sb, wkv, "wkv"),
                                  (wo_sb, wo, "wo")):
            nch = wdst.shape[1]
            for ch in range(nch):
                wst = p1sb.tile([128, wdst.shape[2]], F32, tag="wst", bufs=3,
                                name=f"wst_{wname}_{ch}")
                nc.sync.dma_start(out=wst, in_=wsrc[:, ch, :])
                nc.gpsimd.tensor_copy(out=wdst[:, ch, :], in_=wst)

        def proc_q(i, qp):
            qpv = qp.rearrange("p (h f) -> p h f", h=NH)
            sumsq = p1sb.tile([128, NH], F32, tag="sumsq", bufs=3,
                              name=f"sumsq_{i}")
            sqscr = p1sb.tile([128, BLK], F32, tag="sqscr", bufs=3,
                              name=f"sqscr_{i}")
            nc.scalar.activation(out=sqscr, in_=qp, func=AFT.Square)
            nc.vector.tensor_reduce(
                out=sumsq, in_=sqscr.rearrange("p (h f) -> p h f", h=NH),
                axis=mybir.AxisListType.X, op=mybir.AluOpType.add)
            qsrt = p1sb.tile([128, NH], F32, tag="qsrt", bufs=3,
                             name=f"qsrt_{i}")
            nc.scalar.activation(out=qsrt, in_=sumsq, func=AFT.Sqrt,
                                 bias=eps_col, scale=1.0 / HD)
            rstd = p1sb.tile([128, NH], F32, tag="rstd", bufs=3,
                             name=f"rstd_{i}")
            nc.vector.reciprocal(rstd, qsrt)
            rsc = p1sb.tile([128, NH], F32, tag="rsc", bufs=3,
                            name=f"rsc_{i}")
            nc.vector.tensor_mul(rsc, rstd, qsc_sb)
            rsc_b = rsc[:, :, None].broadcast_to([128, NH, RD])

            qraw = p1sb.tile([128, BLK], F32, tag="qraw", bufs=3,
                             name=f"qraw_{i}")
            nc.scalar.activation(out=qraw, in_=qp, func=AFT.Copy)
            qrv = qraw.rearrange("p (h f) -> p h f", h=NH)
            cos_b = cosq_sb[:, i:i + 1, :].broadcast_to([128, NH, RH])
            sin_b = sinq_sb[:, i:i + 1, :].broadcast_to([128, NH, RH])
            nsin_b = nsinq_sb[:, i:i + 1, :].broadcast_to([128, NH, RH])
            tcq = p1sb.tile([128, NH, RD], F32, tag="tcq", bufs=3,
                            name=f"tcq_{i}")
            tsq = p1sb.tile([128, NH, RD], F32, tag="tsq", bufs=3,
                            name=f"tsq_{i}")
            nc.gpsimd.tensor_mul(tcq[:, :, 0:RH], qrv[:, :, 0:RH], cos_b)
            nc.gpsimd.tensor_mul(tcq[:, :, RH:RD], qrv[:, :, RH:RD], cos_b)
            nc.gpsimd.tensor_mul(tsq[:, :, 0:RH], qrv[:, :, RH:RD], sin_b)
            nc.gpsimd.tensor_mul(tsq[:, :, RH:RD], qrv[:, :, 0:RH], nsin_b)

            qstage = p1sb.tile([128, BLK], F32R, tag="qstage", bufs=3,
                               name=f"qstage_{i}")
            qsv = qstage.rearrange("p (h f) -> p h f", h=NH)
            nc.vector.tensor_add(qsv[:, :, 0:RD], tcq, tsq)
            nc.vector.tensor_mul(qsv[:, :, 0:RD], qsv[:, :, 0:RD], rsc_b)
            nc.vector.tensor_mul(qsv[:, :, RD:HD], qrv[:, :, RD:HD],
                                 rsc[:, :, None].broadcast_to(
                                     [128, NH, HD - RD]))
            for h in range(NH):
                qt_ps = p1ps.tile([128, 128], F32R, tag="misc_ps", bufs=1,
                                  name=f"qtps_{i}_{h}")
                nc.tensor.transpose(qt_ps,
                                    qstage[:, h * 128:(h + 1) * 128],
                                    ident)
                nc.scalar.activation(
                    out=qT_sb[:, h, i * 128:(i + 1) * 128], in_=qt_ps,
                    func=AFT.Copy)

        def proc_k(t, kp):
            ksq = p1sb.tile([128, HD], F32, tag="ksq", bufs=3,
                            name=f"ksq_{t}")
            ksum = p1sb.tile([128, 1], F32, tag="ksum", bufs=3,
                             name=f"ksum_{t}")
            nc.scalar.activation(out=ksq, in_=kp, func=AFT.Square,
                                 accum_out=ksum)
            ksrt = p1sb.tile([128, 1], F32, tag="ksrt", bufs=3,
                             name=f"ksrt_{t}")
            nc.scalar.activation(out=ksrt, in_=ksum, func=AFT.Sqrt,
                                 bias=eps_col, scale=1.0 / HD)
            nc.vector.reciprocal(rstdk_sb[:, t:t + 1], ksrt)
            kraw = p1sb.tile([128, HD], F32, tag="kraw", bufs=3,
                             name=f"kraw_{t}")
            nc.scalar.activation(out=kraw, in_=kp, func=AFT.Copy)
            tckk = p1sb.tile([128, RD], F32, tag="tckk", bufs=3,
                             name=f"tckk_{t}")
            tskk = p1sb.tile([128, RD], F32, tag="tskk", bufs=3,
                             name=f"tskk_{t}")
            nc.gpsimd.tensor_mul(tckk[:, 0:RH], kraw[:, 0:RH],
                                 cosq_sb[:, t, :])
            nc.gpsimd.tensor_mul(tckk[:, RH:RD], kraw[:, RH:RD],
                                 cosq_sb[:, t, :])
            nc.gpsimd.tensor_mul(tskk[:, 0:RH], kraw[:, RH:RD],
                                 sinq_sb[:, t, :])
            nc.gpsimd.tensor_mul(tskk[:, RH:RD], kraw[:, 0:RH],
                                 nsinq_sb[:, t, :])
            kstage = p1sb.tile([128, HD], F32R, tag="kstage", bufs=3,
                               name=f"kstage_{t}")
            nc.vector.tensor_add(kstage[:, 0:RD], tckk, tskk)
            nc.vector.tensor_copy(kstage[:, RD:HD], kraw[:, RD:HD])
            kt_ps = p1ps.tile([128, 128], F32R, tag="misc_ps", bufs=1,
                              name=f"ktps_{t}")
            nc.tensor.transpose(kt_ps, kstage, ident)
            nc.scalar.activation(out=kT_sb[:, t * 128:(t + 1) * 128],
                                 in_=kt_ps, func=AFT.Copy)

        for b in range(NB):
            sl = slice(b * BLK, (b + 1) * BLK)
            xts = []
            for di in range(NDC):
                xt_st = p1sb.tile([128, BLK], F32, tag="xt_st", bufs=3,
                                  name=f"xtst_{b}_{di}")
                nc.sync.dma_start(out=xt_st,
                                  in_=xT[di * 128:(di + 1) * 128, sl])
                xt = p1sb.tile([128, BLK], F32R, tag="xt", bufs=10,
                               name=f"xt_{b}_{di}")
                nc.gpsimd.tensor_copy(out=xt, in_=xt_st)
                xts.append(xt)
            for half in range(2):
                qps2 = [p1ps.tile([128, BLK], F32, tag="q_ps", bufs=4,
                                  name=f"qps_{b}_{half}_{x}")
                        for x in range(2)]
                kvps2 = [p1ps.tile([128, 2 * HD], F32, tag="kv_ps", bufs=2,
                                   name=f"kvps_{b}_{half}_{x}")
                        for x in range(2)]
                for di in range(NDC):
                    st, sp = di == 0, di == NDC - 1
                    for x in range(2):
                        ii = half * 2 + x
                        xsl = xts[di][:, ii * 128:(ii + 1) * 128]
                        mm(qps2[x], lhsT=xsl, rhs=wq_sb[:, di, :],
                           start=st, stop=sp)
                        mm(kvps2[x], lhsT=xsl, rhs=wkv_sb[:, di, :],
                           start=st, stop=sp)
                for x in range(2):
                    i = b * 4 + half * 2 + x
                    t = i
                    proc_k(i, kvps2[x][:, 0:HD])
                    nc.scalar.activation(out=v_sb[:, t, :],
                                         in_=kvps2[x][:, HD:2 * HD],
                                         func=AFT.Copy)
                    proc_q(i, qps2[x])

    # ---- phase 2: attention ----------------------------------------------
    # PSUM: sc_ps [128,1024] x2 (4 banks) + yT_ps x4 (4 banks); the den tail
    # tiles borrow sc_ps slots. Scores/exp are computed per head-PAIR: two
    # matmuls land in one 2-bank psum tile, one Exp covers both.
    with tc.tile_pool(name="p2_psum", bufs=1, space="PSUM") as p2ps, \
         tc.tile_pool(name="p2_sbuf", bufs=1) as p2sb:
        for j in range(NB):
            jsl = slice(j * BLK, (j + 1) * BLK)
            n_t = 4 * j + 4
            yps = [p2ps.tile([128, BLK], F32, tag="yT_ps", bufs=4,
                             name=f"yps_{j}_{h}") for h in range(NH)]
            den_e = [p2sb.tile([128, 2 * BLK], F32, tag="den_e", bufs=3,
                               name=f"dene_{j}_{hp}") for hp in range(2)]
            den_o = [p2sb.tile([128, 2 * BLK], F32, tag="den_o", bufs=3,
                               name=f"deno_{j}_{hp}") for hp in range(2)]
            for t in range(n_t):
                diag = t >= 4 * j
                m = (t - 4 * j) * 128 if diag else 0
                tsl = slice(t * 128, (t + 1) * 128)
                for hp in range(2):
                    sc2 = p2ps.tile([128, 2 * BLK], F32, tag="sc_ps",
                                    bufs=2, name=f"sc2_{j}_{t}_{hp}")
                    expt2 = p2sb.tile([128, 2 * BLK], F32R, tag="expt",
                                      bufs=6, name=f"expt_{j}_{t}_{hp}")
                    for hh in range(2):
                        h = hp * 2 + hh
                        mm(sc2[:, hh * BLK + m:(hh + 1) * BLK],
                           lhsT=kT_sb[:, tsl],
                           rhs=qT_sb[:, h, j * BLK + m:(j + 1) * BLK],
                           start=True, stop=True)
                    sc2v = sc2.rearrange("p (two n) -> p two n", two=2)
                    e2v = expt2.rearrange("p (two n) -> p two n", two=2)
                    nc.scalar.activation(out=e2v[:, :, m:BLK],
                                         in_=sc2v[:, :, m:BLK], func=AFT.Exp,
                                         scale=rstdk_sb[:, t:t + 1])
                    if diag:
                        nc.vector.tensor_mul(
                            e2v[:, :, m:m + 128], e2v[:, :, m:m + 128],
                            tri[:, None, :].broadcast_to([128, 2, 128]))
                    ev_m = e2v[:, :, m:BLK]
                    de_v = den_e[hp].rearrange("p (two n) -> p two n", two=2)
                    do_v = den_o[hp].rearrange("p (two n) -> p two n", two=2)
                    if t == 0:
                        nc.vector.tensor_copy(den_e[hp], expt2)
                    elif j == 0 or t % 2 == 0:
                        nc.vector.tensor_add(de_v[:, :, m:BLK],
                                             de_v[:, :, m:BLK], ev_m)
                    elif t == 1:
                        nc.gpsimd.tensor_copy(den_o[hp], expt2)
                    else:
                        nc.gpsimd.tensor_add(do_v[:, :, m:BLK],
                                             do_v[:, :, m:BLK], ev_m)
                    for hh in range(2):
                        h = hp * 2 + hh
                        esl = expt2[:, hh * BLK + m:(hh + 1) * BLK]
                        mm(yps[h][:, m:BLK], lhsT=v_sb[:, t, :], rhs=esl,
                           start=(t == 0), stop=(t == n_t - 1))
            den_f = []
            for hp in range(2):
                df = p2sb.tile([128, 2 * BLK], F32R, tag="den_f", bufs=2,
                               name=f"denf_{j}_{hp}")
                if j > 0:
                    nc.vector.tensor_add(df, den_e[hp], den_o[hp])
                else:
                    nc.vector.tensor_copy(df, den_e[hp])
                den_f.append(df)
            for h in range(NH):
                hp, hh = divmod(h, 2)
                dn_ps = p2ps.tile([1, BLK], F32, tag="sc_ps", bufs=2,
                                  name=f"dnps_{j}_{h}")
                mm(dn_ps, lhsT=ones_col,
                   rhs=den_f[hp][:, hh * BLK:(hh + 1) * BLK],
                   start=True, stop=True)
                rdr = p2sb.tile([1, BLK], F32R, tag="rdr", bufs=4,
                                name=f"rdr_{j}_{h}")
                nc.vector.reciprocal(rdr, dn_ps)
                rdb_ps = p2ps.tile([128, BLK], F32, tag="sc_ps", bufs=2,
                                   name=f"rdbps_{j}_{h}")
                mm(rdb_ps, lhsT=ones_row, rhs=rdr,
                   start=True, stop=True)
                rdb_sb = p2sb.tile([128, BLK], F32, tag="rdb_sb", bufs=3,
                                   name=f"rdbsb_{j}_{h}")
                nc.scalar.activation(out=rdb_sb, in_=rdb_ps, func=AFT.Copy)
                nc.vector.tensor_mul(yT_sb[:, h, jsl], yps[h], rdb_sb)

    # ---- phase 3: output projection --------------------------------------
    with tc.tile_pool(name="p3_psum", bufs=1, space="PSUM") as p3ps, \
         tc.tile_pool(name="p3_sbuf", bufs=1) as p3sb:
        for oc in range(8):
            osl = slice(oc * 128, (oc + 1) * 128)
            ops_ = [p3ps.tile([128, BLK], F32, tag="out_ps", bufs=8,
                              name=f"ops_{oc}_{j}") for j in range(NB)]
            for c in range(NH):
                for j in range(NB):
                    mm(ops_[j], lhsT=wo_sb[:, c, osl],
                       rhs=yT_sb[:, c, j * BLK:(j + 1) * BLK],
                       start=(c == 0), stop=(c == NH - 1))
            for j in range(NB):
                oc_sb = p3sb.tile([128, BLK], F32, tag="oc_sb", bufs=8,
                                  name=f"ocsb_{oc}_{j}")
                nc.vector.tensor_copy(out=oc_sb, in_=ops_[j])
                nc.sync.dma_start(out=outT[osl, j * BLK:(j + 1) * BLK],
                                  in_=oc_sb)

    persist_cm.__exit__(None, None, None)


_NC_CACHE = {}


def _get_nc():
    if "nc" not in _NC_CACHE:
        _NC_CACHE["nc"] = _build_nc()
    return _NC_CACHE["nc"]


def _host_tables():
    pos = np.arange(S, dtype=np.float32)
    inv = (1.0 / (10000.0 ** (np.arange(0, RD, 2, dtype=np.float32) / RD)))
    fr = np.outer(pos, inv).astype(np.float32)          # [S, 32]
    cos, sin = np.cos(fr), np.sin(fr)
    tile128 = lambda a: np.ascontiguousarray(
        a.reshape(NT, 128, RH).transpose(1, 0, 2))      # [128, NT, 32]
    cosq = tile128(cos)
    sinq = tile128(sin)
    nsinq = tile128(-sin)
    cosk2 = np.ascontiguousarray(np.vstack([cos.T, cos.T]))   # [64, S]
    sk2 = np.ascontiguousarray(np.vstack([sin.T, -sin.T]))    # [64, S]
    return cosq, sinq, nsinq, cosk2, sk2


def kernel(x, w_q, w_k, w_v, w_o, q_gain):
    x = np.asarray(x, dtype=np.float32)
    w_q = np.asarray(w_q, dtype=np.float32)
    w_k = np.asarray(w_k, dtype=np.float32)
    w_v = np.asarray(w_v, dtype=np.float32)
    w_o = np.asarray(w_o, dtype=np.float32)
    q_gain = np.asarray(q_gain, dtype=np.float32)

    nc = _get_nc()
    cosq, sinq, nsinq, cosk2, sk2 = _host_tables()

    def wtile(wT, chunks, width):
        # [chunks*128, width] -> [128, chunks, width]
        return np.ascontiguousarray(
            wT.reshape(chunks, 128, width).transpose(1, 0, 2))

    in_maps = []
    for core in range(8):
        b, g = divmod(core, 2)
        cols = slice(g * NH * HD, (g + 1) * NH * HD)
        xTc = np.ascontiguousarray(x[b].T)                       # [D, S]
        wq_t = wtile(np.ascontiguousarray(w_q[cols, :].T), NDC, NH * HD)
        wkv_t = wtile(np.ascontiguousarray(np.concatenate(
            [w_k[g * HD:(g + 1) * HD, :].T, w_v[g * HD:(g + 1) * HD, :].T],
            axis=1)), NDC, 2 * HD)
        wo_t = wtile(np.ascontiguousarray(w_o[:, cols].T), NH, D)
        qsc_h = (q_gain[g * NH:(g + 1) * NH] *
                 np.float32(HD ** -0.5)).astype(np.float32).reshape(1, NH)
        in_maps.append(dict(
            xT=xTc, wq=wq_t, wkv=wkv_t, wo=wo_t,
            cosq=cosq, sinq=sinq, nsinq=nsinq, cosk2=cosk2, sk2=sk2,
            qsc=qsc_h))

    res = bass_utils.run_bass_kernel_spmd(nc, in_maps,
                                          core_ids=list(range(8)))
    out = np.empty((B, S, D), dtype=np.float32)
    for b in range(B):
        p0 = res.results[2 * b]["outT"]
        p1 = res.results[2 * b + 1]["outT"]
        out[b] = (p0 + p1).T
    return out



# revision 14
# speedup vs baseline: 2.0270x; 2.0270x over previous
"""Trainium2 Bass kernel for CausalSelfAttention (GQA + RMSNorm + partial RoPE).

Sharding: 8 cores = (batch b in 0..3) x (kv-head group g in 0..1).
Each core computes the full attention for its (b, g) slice and the partial
output projection over its head columns; the host sums the two partials per
batch and transposes back ([o, s] -> [s, o]).

v2 layout strategy (per core, S=2048, D=1024, HD=128, 4 q heads, 1 kv):
  - All matmul operands in bf16 (x and weights converted host-side; halves
    HBM traffic and runs 1 cyc/row on PE without f32r rounding copies).
  - QKV projections from xT [d, s] tiles; q in [s, o] layout for
    RMS-norm/RoPE (free-dim reductions), then scaled by rstd*gain*HD^-0.5
    and moved to qT [hd, h, s] via XBAR DMA transpose (off the PE).
  - k rope'd in [s, hd], moved to kT [hd, s] by DMA transpose; k's rstd is
    folded into the exp() per-partition scale.
  - scoresT [sk, sq] per head-pair psum; exp on ACT; causal via restricted
    matmul ranges + one triangular mask on the diagonal 128 block.
  - attention runs head-pair serialized (yps = 2 psum banks) so the output
    projection for block j-1 interleaves with attention block j on the PE.
  - softmax denominator: bf16 running sum on DVE (4x mode), cross-partition
    reduce + broadcast on GPSIMD, reciprocal on DVE; applied in-place to
    yT after psum eviction so psum banks never wait on the divide chain.
"""

import sys

for _p in ("/opt/trn_rl_repo",):
    if _p not in sys.path:
        sys.path.insert(0, _p)

import numpy as np
import ml_dtypes

import concourse.bass as bass
import concourse.bacc as bacc
import concourse.mybir as mybir
import concourse.tile as tile
from concourse import bass_utils
from concourse import bass_isa

F32 = mybir.dt.float32
BF16 = mybir.dt.bfloat16
AFT = mybir.ActivationFunctionType
BF16NP = ml_dtypes.bfloat16

B, S, D = 4, 2048, 1024
H, KVH, HD = 8, 2, 128
NH = H // KVH          # q heads per core = 4
RD, RH = 64, 32        # rope dims / half
NB, BLK = 4, 512       # s blocks
NT, TS = 16, 128       # s tiles
NDC = D // 128         # 8 d-chunks
EPS = float(np.finfo(np.float32).eps)


def _build_nc(reps=1):
    nc = bacc.Bacc("TRN2", target_bir_lowering=False, debug=False,
                   enable_asserts=False)

    xT = nc.dram_tensor("xT", (D, S), BF16, kind="ExternalInput").ap()
    wq = nc.dram_tensor("wq", (128, NDC, NH * HD), BF16,
                        kind="ExternalInput").ap()
    wkv = nc.dram_tensor("wkv", (128, NDC, 2 * HD), BF16,
                         kind="ExternalInput").ap()
    wo = nc.dram_tensor("wo", (128, NH, D), BF16, kind="ExternalInput").ap()
    cosq = nc.dram_tensor("cosq", (128, NT, RH), BF16,
                          kind="ExternalInput").ap()
    sinq = nc.dram_tensor("sinq", (128, NT, RH), BF16,
                          kind="ExternalInput").ap()
    nsinq = nc.dram_tensor("nsinq", (128, NT, RH), BF16,
                           kind="ExternalInput").ap()
    qsc = nc.dram_tensor("qsc", (1, NH), F32, kind="ExternalInput").ap()
    outT = nc.dram_tensor("outT", (D, S), F32, kind="ExternalOutput").ap()

    with tile.TileContext(nc) as tc, \
         nc.allow_low_precision(reason="bf16 attention pipeline"):
        from concourse import library_config
        nc.gpsimd.load_library(library_config.attn)
        for _rep in range(reps):
            _kern(nc, tc, xT, wq, wkv, wo, cosq, sinq, nsinq, qsc, outT)
    nc.compile()
    return nc


def _kern(nc, tc, xT, wq, wkv, wo, cosq, sinq, nsinq, qsc, outT):
    import os
    _PH = int(os.environ.get("KPH", "3"))
    mm = nc.tensor.matmul

    persist_cm = tc.tile_pool(name="persist", bufs=1)
    persist = persist_cm.__enter__()
    # ---- persistent tiles -------------------------------------------------
    wq_sb = persist.tile([128, NDC, NH * HD], BF16, tag="wq_sb", name="wq_sb")
    nc.sync.dma_start(out=wq_sb, in_=wq)
    wkv_sb = persist.tile([128, NDC, 2 * HD], BF16, tag="wkv_sb",
                          name="wkv_sb")
    nc.sync.dma_start(out=wkv_sb, in_=wkv)
    wo_sb = persist.tile([128, NH, D], BF16, tag="wo_sb", name="wo_sb")
    nc.sync.dma_start(out=wo_sb, in_=wo)
    cosq_sb = persist.tile([128, NT, RH], BF16, tag="cosq_sb", name="cosq_sb")
    nc.sync.dma_start(out=cosq_sb, in_=cosq)
    sinq_sb = persist.tile([128, NT, RH], BF16, tag="sinq_sb", name="sinq_sb")
    nc.sync.dma_start(out=sinq_sb, in_=sinq)
    nsinq_sb = persist.tile([128, NT, RH], BF16, tag="nsinq_sb",
                            name="nsinq_sb")
    nc.sync.dma_start(out=nsinq_sb, in_=nsinq)
    qsc_sb = persist.tile([128, NH], F32, tag="qsc_sb", name="qsc_sb")
    nc.sync.dma_start(out=qsc_sb, in_=qsc.to_broadcast((128, NH)))

    eps_col = persist.tile([128, 1], F32, tag="eps_col", name="eps_col")
    nc.vector.memset(eps_col, EPS)
    # tri[r, c] = 1.0 if r <= c else 0.0  (causal keep-mask on the diagonal)
    tri_f = persist.tile([128, 128], F32, tag="tri_f", name="tri_f")
    nc.gpsimd.memset(tri_f, 1.0)
    nc.gpsimd.affine_select(
        out=tri_f, in_=tri_f, compare_op=mybir.AluOpType.is_ge, fill=0.0,
        base=0, pattern=[[1, 128]], channel_multiplier=-1)
    tri = persist.tile([128, 128], BF16, tag="tri", name="tri")
    nc.vector.tensor_copy(out=tri, in_=tri_f)

    qT_sb = persist.tile([128, NH, S], BF16, tag="qT_sb", name="qT_sb")
    kT_sb = persist.tile([128, S], BF16, tag="kT_sb", name="kT_sb")
    v_sb = persist.tile([128, NT, HD], BF16, tag="v_sb", name="v_sb")
    rstdk_sb = persist.tile([128, NT], F32, tag="rstdk_sb", name="rstdk_sb")
    yT_sb = persist.tile([128, NH, S], BF16, tag="yT_sb", name="yT_sb")

    # ---- phase 1: projections + norm + rope + DMA transposes --------------
    # PSUM budget: q_ps 4 banks + kv_ps 2 = 6.
    with tc.tile_pool(name="p1_psum", bufs=1, space="PSUM") as p1ps, \
         tc.tile_pool(name="p1_sbuf", bufs=1) as p1sb:

        def proc_q(i, qp):
            # raw q (bf16) for stats + rope
            qraw = p1sb.tile([128, NH, HD], BF16, tag="qraw", bufs=3,
                             name=f"qraw_{i}")
            nc.scalar.activation(out=qraw.rearrange("p h f -> p (h f)"),
                                 in_=qp, func=AFT.Copy)
            # per-head sum of squares -> rstd -> rsc = rstd * gain * HD^-.5
            sumsq = p1sb.tile([128, NH], F32, tag="sumsq", bufs=3,
                              name=f"sumsq_{i}")
            sqf = p1sb.tile([128, NH, HD], BF16, tag="sqf", bufs=3,
                            name=f"sqf_{i}")
            nc.vector.tensor_mul(sqf, qraw, qraw)
            nc.vector.tensor_reduce(
                out=sumsq, in_=sqf, axis=mybir.AxisListType.X,
                op=mybir.AluOpType.add)
            qsrt = p1sb.tile([128, NH], F32, tag="qsrt", bufs=3,
                             name=f"qsrt_{i}")
            nc.scalar.activation(out=qsrt, in_=sumsq, func=AFT.Sqrt,
                                 bias=eps_col, scale=1.0 / HD)
            rstd = p1sb.tile([128, NH], F32, tag="rstd", bufs=3,
                             name=f"rstd_{i}")
            nc.vector.reciprocal(rstd, qsrt)
            rsc = p1sb.tile([128, NH], F32, tag="rsc", bufs=3,
                            name=f"rsc_{i}")
            nc.vector.tensor_mul(rsc, rstd, qsc_sb)

            # qstage = q * rsc (per head), then rope the first RD dims
            qstage = p1sb.tile([128, NH, HD], BF16, tag="qstage", bufs=3,
                               name=f"qstage_{i}")
            for h in range(NH):
                nc.vector.tensor_scalar_mul(qstage[:, h, :], qraw[:, h, :],
                                            rsc[:, h:h + 1])
            cos_b = cosq_sb[:, i:i + 1, :].broadcast_to([128, NH, RH])
            sin_b = sinq_sb[:, i:i + 1, :].broadcast_to([128, NH, RH])
            nsin_b = nsinq_sb[:, i:i + 1, :].broadcast_to([128, NH, RH])
            tcq = p1sb.tile([128, NH, RD], BF16, tag="tcq", bufs=3,
                            name=f"tcq_{i}")
            tsq = p1sb.tile([128, NH, RD], BF16, tag="tsq", bufs=3,
                            name=f"tsq_{i}")
            nc.vector.tensor_mul(tcq[:, :, 0:RH], qstage[:, :, 0:RH], cos_b)
            nc.vector.tensor_mul(tcq[:, :, RH:RD], qstage[:, :, RH:RD], cos_b)
            nc.vector.tensor_mul(tsq[:, :, 0:RH], qstage[:, :, RH:RD], sin_b)
            nc.vector.tensor_mul(tsq[:, :, RH:RD], qstage[:, :, 0:RH], nsin_b)
            nc.vector.tensor_add(qstage[:, :, 0:RD], tcq, tsq)
            # qT[hd, h, s-tile] = qstage[s, h, hd]
            nc.sync.dma_start_transpose(
                out=qT_sb[:, :, i * 128:(i + 1) * 128],
                in_=qstage.rearrange("p h f -> p (h f)"))

        def proc_k(t, kp):
            kraw = p1sb.tile([128, HD], BF16, tag="kraw", bufs=3,
                             name=f"kraw_{t}")
            nc.scalar.activation(out=kraw, in_=kp, func=AFT.Copy)
            ksum = p1sb.tile([128, 1], F32, tag="ksum", bufs=3,
                             name=f"ksum_{t}")
            ksq = p1sb.tile([128, HD], BF16, tag="ksq", bufs=3,
                            name=f"ksq_{t}")
            nc.vector.tensor_mul(ksq, kraw, kraw)
            nc.vector.tensor_reduce(
                out=ksum, in_=ksq[:, None, :], axis=mybir.AxisListType.X,
                op=mybir.AluOpType.add)
            ksrt = p1sb.tile([128, 1], F32, tag="ksrt", bufs=3,
                             name=f"ksrt_{t}")
            nc.scalar.activation(out=ksrt, in_=ksum, func=AFT.Sqrt,
                                 bias=eps_col, scale=1.0 / HD)
            nc.vector.reciprocal(rstdk_sb[:, t:t + 1], ksrt)
            kstage = p1sb.tile([128, HD], BF16, tag="kstage", bufs=3,
                               name=f"kstage_{t}")
            tck = p1sb.tile([128, RD], BF16, tag="tck", bufs=3,
                            name=f"tck_{t}")
            tsk = p1sb.tile([128, RD], BF16, tag="tsk", bufs=3,
                            name=f"tsk_{t}")
            nc.vector.tensor_mul(tck[:, 0:RH], kraw[:, 0:RH],
                                 cosq_sb[:, t, :])
            nc.vector.tensor_mul(tck[:, RH:RD], kraw[:, RH:RD],
                                 cosq_sb[:, t, :])
            nc.vector.tensor_mul(tsk[:, 0:RH], kraw[:, RH:RD],
                                 sinq_sb[:, t, :])
            nc.vector.tensor_mul(tsk[:, RH:RD], kraw[:, 0:RH],
                                 nsinq_sb[:, t, :])
            nc.vector.tensor_add(kstage[:, 0:RD], tck, tsk)
            nc.vector.tensor_copy(kstage[:, RD:HD], kraw[:, RD:HD])
            nc.sync.dma_start_transpose(
                out=kT_sb[:, t * 128:(t + 1) * 128], in_=kstage)

        for b in range(NB):
            sl = slice(b * BLK, (b + 1) * BLK)
            xts = []
            for di in range(NDC):
                xt = p1sb.tile([128, BLK], BF16, tag="xt", bufs=10,
                               name=f"xt_{b}_{di}")
                nc.sync.dma_start(out=xt,
                                  in_=xT[di * 128:(di + 1) * 128, sl])
                xts.append(xt)
            for half in range(2):
                qps2 = [p1ps.tile([128, BLK], F32, tag="q_ps", bufs=4,
                                  name=f"qps_{b}_{half}_{x}")
                        for x in range(2)]
                kvps2 = [p1ps.tile([128, 2 * HD], F32, tag="kv_ps", bufs=2,
                                   name=f"kvps_{b}_{half}_{x}")
                        for x in range(2)]
                for di in range(NDC):
                    st, sp = di == 0, di == NDC - 1
                    for x in range(2):
                        ii = half * 2 + x
                        xsl = xts[di][:, ii * 128:(ii + 1) * 128]
                        mm(qps2[x], lhsT=xsl, rhs=wq_sb[:, di, :],
                           start=st, stop=sp)
                        mm(kvps2[x], lhsT=xsl, rhs=wkv_sb[:, di, :],
                           start=st, stop=sp)
                for x in range(2):
                    i = b * 4 + half * 2 + x
                    proc_k(i, kvps2[x][:, 0:HD])
                    nc.scalar.activation(out=v_sb[:, i, :],
                                         in_=kvps2[x][:, HD:2 * HD],
                                         func=AFT.Copy)
                    proc_q(i, qps2[x])

    if _PH < 2:
        with tc.tile_pool(name="px_sbuf", bufs=1) as pxsb:
            z = pxsb.tile([128, BLK], F32, tag="z", name="zz")
            nc.vector.memset(z, 0.0)
            for oc in range(8):
                for jj in range(NB):
                    nc.sync.dma_start(
                        out=outT[oc * 128:(oc + 1) * 128,
                                 jj * BLK:(jj + 1) * BLK], in_=z)
        persist_cm.__exit__(None, None, None)
        return

    # ---- phase 2+3: attention with interleaved output projection ---------
    # PSUM: sc_ps [128,1024] x2 (4 banks) + y_ps x2 (2) + out_ps x2 (2) = 8.
    # Heads processed in pairs (hp serialized) so only 2 y psum banks live;
    # out-projection units for block j-1 are interleaved into block j's
    # attention to keep the PE fed while ACT runs the exps.
    with tc.tile_pool(name="p2_psum", bufs=1, space="PSUM") as p2ps, \
         tc.tile_pool(name="p2_sbuf", bufs=1) as p2sb:

        def p3_unit(j, oc):
            osl = slice(oc * 128, (oc + 1) * 128)
            ops = p2ps.tile([128, BLK], F32, tag="out_ps", bufs=2,
                            name=f"ops_{j}_{oc}")
            for c in range(NH):
                mm(ops, lhsT=wo_sb[:, c, osl],
                   rhs=yT_sb[:, c, j * BLK:(j + 1) * BLK],
                   start=(c == 0), stop=(c == NH - 1))
            oc_sb = p2sb.tile([128, BLK], F32, tag="oc_sb", bufs=4,
                              name=f"ocsb_{j}_{oc}")
            if oc % 2 == 0:
                nc.vector.tensor_copy(out=oc_sb, in_=ops)
            else:
                nc.scalar.activation(out=oc_sb, in_=ops, func=AFT.Copy)
            nc.sync.dma_start(out=outT[osl, j * BLK:(j + 1) * BLK],
                              in_=oc_sb)

        for j in range(NB):
            jsl = slice(j * BLK, (j + 1) * BLK)
            n_t = 4 * j + 4
            p3q = list(range(8)) if j > 0 else []  # oc units of block j-1
            for hp in range(2):
                yps = [p2ps.tile([128, BLK], F32, tag="y_ps", bufs=2,
                                 name=f"yps_{j}_{hp}_{hh}")
                       for hh in range(2)]
                den = p2sb.tile([128, 2, BLK], BF16, tag="den", bufs=3,
                                name=f"den_{j}_{hp}")
                for t in range(n_t):
                    diag = t >= 4 * j
                    m = (t - 4 * j) * 128 if diag else 0
                    tsl = slice(t * 128, (t + 1) * 128)
                    sc2 = p2ps.tile([128, 2, BLK], F32, tag="sc_ps",
                                    bufs=2, name=f"sc2_{j}_{hp}_{t}")
                    expt = p2sb.tile([128, 2, BLK], BF16, tag="expt",
                                     bufs=6, name=f"expt_{j}_{hp}_{t}")
                    for hh in range(2):
                        h = hp * 2 + hh
                        mm(sc2[:, hh, m:BLK],
                           lhsT=kT_sb[:, tsl],
                           rhs=qT_sb[:, h, j * BLK + m:(j + 1) * BLK],
                           start=True, stop=True)
                    nc.scalar.activation(out=expt[:, :, m:BLK],
                                         in_=sc2[:, :, m:BLK], func=AFT.Exp,
                                         scale=rstdk_sb[:, t:t + 1])
                    if diag:
                        nc.vector.tensor_mul(
                            expt[:, :, m:m + 128], expt[:, :, m:m + 128],
                            tri[:, None, :].broadcast_to([128, 2, 128]))
                    if t == 0:
                        nc.vector.tensor_copy(den, expt)
                    else:
                        nc.vector.tensor_add(den[:, :, m:BLK],
                                             den[:, :, m:BLK],
                                             expt[:, :, m:BLK])
                    for hh in range(2):
                        mm(yps[hh][:, m:BLK], lhsT=v_sb[:, t, :],
                           rhs=expt[:, hh, m:BLK],
                           start=(t == 0), stop=(t == n_t - 1))
                    if p3q and t % 2 == 1:
                        p3_unit(j - 1, p3q.pop(0))
                # evict y psum (unscaled) so banks don't wait on the divide
                for hh in range(2):
                    h = hp * 2 + hh
                    if hh == 0:
                        nc.vector.tensor_copy(out=yT_sb[:, h, jsl],
                                              in_=yps[hh])
                    else:
                        nc.scalar.activation(out=yT_sb[:, h, jsl],
                                             in_=yps[hh], func=AFT.Copy)
                # denominator: fused cross-partition sum + broadcast -> 1/x
                dnb = p2sb.tile([128, 2, BLK], BF16, tag="dnb", bufs=3,
                                name=f"dnb_{j}_{hp}")
                nc.gpsimd.partition_all_reduce(dnb, den, channels=128,
                                               reduce_op=bass_isa.ReduceOp.add)
                rdb = p2sb.tile([128, 2, BLK], BF16, tag="rdb", bufs=3,
                                name=f"rdb_{j}_{hp}")
                nc.vector.reciprocal(rdb, dnb)
                for hh in range(2):
                    h = hp * 2 + hh
                    nc.vector.tensor_mul(yT_sb[:, h, jsl], yT_sb[:, h, jsl],
                                         rdb[:, hh, :])
            while p3q:
                p3_unit(j - 1, p3q.pop(0))
        for oc in range(8):
            p3_unit(NB - 1, oc)

    persist_cm.__exit__(None, None, None)


_NC_CACHE = {}


def _get_nc():
    if "nc" not in _NC_CACHE:
        _NC_CACHE["nc"] = _build_nc()
    return _NC_CACHE["nc"]


def _host_tables():
    pos = np.arange(S, dtype=np.float32)
    inv = (1.0 / (10000.0 ** (np.arange(0, RD, 2, dtype=np.float32) / RD)))
    fr = np.outer(pos, inv).astype(np.float32)          # [S, 32]
    cos, sin = np.cos(fr), np.sin(fr)
    tile128 = lambda a: np.ascontiguousarray(
        a.reshape(NT, 128, RH).transpose(1, 0, 2)).astype(BF16NP)
    return tile128(cos), tile128(sin), tile128(-sin)


def kernel(x, w_q, w_k, w_v, w_o, q_gain):
    x = np.asarray(x, dtype=np.float32)
    w_q = np.asarray(w_q, dtype=np.float32)
    w_k = np.asarray(w_k, dtype=np.float32)
    w_v = np.asarray(w_v, dtype=np.float32)
    w_o = np.asarray(w_o, dtype=np.float32)
    q_gain = np.asarray(q_gain, dtype=np.float32)

    nc = _get_nc()
    cosq, sinq, nsinq = _host_tables()

    def wtile(wT, chunks, width):
        # [chunks*128, width] -> [128, chunks, width] bf16
        return np.ascontiguousarray(
            wT.reshape(chunks, 128, width).transpose(1, 0, 2)).astype(BF16NP)

    in_maps = []
    for core in range(8):
        b, g = divmod(core, 2)
        cols = slice(g * NH * HD, (g + 1) * NH * HD)
        xTc = np.ascontiguousarray(x[b].T).astype(BF16NP)        # [D, S]
        wq_t = wtile(np.ascontiguousarray(w_q[cols, :].T), NDC, NH * HD)
        wkv_t = wtile(np.ascontiguousarray(np.concatenate(
            [w_k[g * HD:(g + 1) * HD, :].T, w_v[g * HD:(g + 1) * HD, :].T],
            axis=1)), NDC, 2 * HD)
        wo_t = wtile(np.ascontiguousarray(w_o[:, cols].T), NH, D)
        qsc_h = (q_gain[g * NH:(g + 1) * NH] *
                 np.float32(HD ** -0.5)).astype(np.float32).reshape(1, NH)
        in_maps.append(dict(
            xT=xTc, wq=wq_t, wkv=wkv_t, wo=wo_t,
            cosq=cosq, sinq=sinq, nsinq=nsinq, qsc=qsc_h))

    res = bass_utils.run_bass_kernel_spmd(nc, in_maps,
                                          core_ids=list(range(8)))
    out = np.empty((B, S, D), dtype=np.float32)
    for b in range(B):
        p0 = res.results[2 * b]["outT"]
        p1 = res.results[2 * b + 1]["outT"]
        out[b] = (p0 + p1).T
    return out
